# revision 4
# baseline (speedup 1.0000x reference)
"""Multi-head attention + output projection, sharded over 8 TRN2 NeuronCores.

Problem: Q,K,V [4,1024,1024] f32; 16 heads x 64 dim; softmax(QK^T/sqrt(1024))V,
concat heads, out @ W_H.T + b_H.

Sharding: 8 cores = 4 batch x 2 query-halves. Each core computes full attention
(all 16 heads, all 1024 keys) for its 512 queries plus the output projection for
those rows. Output rows are disjoint -> no collectives.

v2 per-core kernel (bf16 operands, fp32 PSUM accumulation):
Heads processed in pairs (2j at partitions 0-63, 2j+1 at 64-127). Per key
chunk c (128 keys): two QK^T matmuls on complementary PE row groups (contract
64 each -> concurrent on HW), one exp over both heads' scores (ACT, FD=1024,
PSUM->SBUF bf16), then two attn*V matmuls accumulating into [65,512] columns
(V carries a ones column -> row 64 = softmax denominator).
Normalization: DVE reciprocal of the two denominator rows, PE matmul broadcast
(ones[1,64].T @ recip[1,512]) into a rotating PSUM slot, DVE multiply. Odd-head
outputs move to partitions 64-127 of the proj lhsT via a gpsimd SBUF DMA.
Projection: [128q,512n] tiles accumulating 8 chunk matmuls + bias add + store.
W_H/bias DMAs ride the scalar-engine HWDGE ring so they never delay K/Q/V.
"""
import sys
import os

sys.path.insert(0, "/opt/trn_rl_repo")

import numpy as np

B, L, D, H, HD = 4, 1024, 1024, 16, 64
NCORES = 8
QBLK = L // 2  # 512 queries per core
SCALE = 1.0 / np.sqrt(np.float32(D))

_STATE = {}


def _build_nc(niter=1, ablate="full"):
    import concourse.bass as bass
    import concourse.tile as tile
    from concourse import bacc, mybir
    from contextlib import ExitStack

    F32 = mybir.dt.float32
    F32R = mybir.dt.float32r
    BF16 = mybir.dt.bfloat16
    Exp = mybir.ActivationFunctionType.Exp

    nc = bacc.Bacc("TRN2", target_bir_lowering=False, debug=False, use_seq_codegen=True)
    qt = nc.dram_tensor("qt", [128, 8, QBLK], BF16, kind="ExternalInput")
    kt = nc.dram_tensor("kt", [128, 8, L], BF16, kind="ExternalInput")
    vv = nc.dram_tensor("vv", [128, 8, 2, 8, HD + 1], BF16, kind="ExternalInput")
    wht = nc.dram_tensor("wht", [128, 8, D], BF16, kind="ExternalInput")
    bias = nc.dram_tensor("bias", [128, D], F32, kind="ExternalInput")
    out = nc.dram_tensor("out", [QBLK, D], F32, kind="ExternalOutput")

    with tile.TileContext(nc) as tc, ExitStack() as ctx:
        singles = ctx.enter_context(tc.tile_pool(name="singles", bufs=1))
        qk_pool = ctx.enter_context(tc.tile_pool(name="qk", bufs=2))
        v_pool = ctx.enter_context(tc.tile_pool(name="vp", bufs=2))
        exp_pool = ctx.enter_context(tc.tile_pool(name="exp", bufs=2))
        norm_pool = ctx.enter_context(tc.tile_pool(name="norm", bufs=2))
        final_pool = ctx.enter_context(tc.tile_pool(name="final", bufs=2))
        # scps slots ([128,2,512] = 2 banks x2 bufs) also rotate the recip-
        # broadcast tiles and the projection accumulators via tag sharing.
        scps = ctx.enter_context(tc.tile_pool(name="scps", bufs=2, space="PSUM"))
        ov_ps = ctx.enter_context(tc.tile_pool(name="ovps", bufs=2, space="PSUM"))

        def body(_=None):
            # warm the exp table while the first DMAs run
            warm_in = singles.tile([1, 8], F32, tag="warm_in")
            warm_out = singles.tile([1, 8], F32, tag="warm_out")
            nc.vector.memset(warm_in, 0.0)
            nc.scalar.activation(out=warm_out, in_=warm_in, func=Exp)

            # normalized concatenated attention output (proj lhsT): [hd, q]
            outT = singles.tile([128, 8, QBLK], BF16, tag="outT")
            sb_bias = singles.tile([128, D], F32, tag="bias")
            sb_wht = singles.tile([128, 8, D], BF16, tag="wht")
            # staged first-half projection partial sums (incl. bias): [q,jn,n]
            p1sb = singles.tile([128, 4, 2, 512], F32, tag="p1sb")

            for j in range(8):  # head pair (2j, 2j+1)
                kt_t = qk_pool.tile([128, L], BF16, tag="kt")
                nc.sync.dma_start(kt_t, kt.ap()[:, j])
                qt_t = qk_pool.tile([128, QBLK], BF16, tag="qt")
                nc.sync.dma_start(qt_t, qt.ap()[:, j])
                v_t = v_pool.tile([128, 2, 8, HD + 1], BF16, tag="v")
                nc.sync.dma_start(v_t, vv.ap()[:, j])
                # big cold tensors ride the ACT HWDGE ring in per-pair chunks
                # so they never monopolize the shared SDMA engines
                nc.scalar.dma_start(sb_wht[:, j], wht.ap()[:, j])
                if j == 1:
                    nc.scalar.dma_start(sb_bias, bias.ap())

                expT = exp_pool.tile([128, 8, 2, QBLK], BF16, tag="expT")
                ov = ov_ps.tile([128, 2, QBLK], F32, tag="ov")

                for c in range(8):  # key chunk (128 keys)
                    S = scps.tile([128, 2, QBLK], F32, tag="S")
                    nc.tensor.matmul(
                        S[:, 0, :], lhsT=kt_t[0:HD, c * 128:(c + 1) * 128],
                        rhs=qt_t[0:HD, :], start=True, stop=True)
                    nc.tensor.matmul(
                        S[:, 1, :], lhsT=kt_t[HD:128, c * 128:(c + 1) * 128],
                        rhs=qt_t[HD:128, :], start=True, stop=True)
                    nc.scalar.activation(out=expT[:, c, :, :], in_=S, func=Exp)
                    for par in range(2):
                        nc.tensor.matmul(
                            ov[0:HD + 1, par, :], lhsT=v_t[:, par, c, :],
                            rhs=expT[:, c, par, :],
                            start=(c == 0), stop=(c == 7))

                # softmax denominators -> reciprocal -> gpsimd broadcast
                # across the 64 hd partitions -> multiply
                recip = norm_pool.tile([1, 2, QBLK], F32, tag="recip")
                bc_sb = norm_pool.tile([HD, 2, QBLK], F32, tag="bcsb")
                tmp = norm_pool.tile([HD, QBLK], BF16, tag="tmp")
                if j < 7:
                    nc.vector.reciprocal(out=recip, in_=ov[HD:HD + 1, :, :])
                    nc.gpsimd.partition_broadcast(bc_sb, recip[0:1, :, :])
                    nc.vector.tensor_mul(
                        out=tmp, in0=ov[0:HD, 1, :], in1=bc_sb[:, 1, :])
                    nc.scalar.dma_start(outT[HD:128, j, :], tmp)
                    nc.vector.tensor_mul(
                        out=outT[0:HD, j, :], in0=ov[0:HD, 0, :],
                        in1=bc_sb[:, 0, :])
                else:
                    # last pair: shortest path to the odd-head move, since
                    # the projection tail's chunk-7 matmuls wait on it
                    nc.vector.reciprocal(
                        out=recip[:, 1, :], in_=ov[HD:HD + 1, 1, :])
                    nc.gpsimd.partition_broadcast(
                        bc_sb[:, 1, :], recip[0:1, 1, :])
                    nc.vector.reciprocal(
                        out=recip[:, 0, :], in_=ov[HD:HD + 1, 0, :])
                    nc.gpsimd.partition_broadcast(
                        bc_sb[:, 0, :], recip[0:1, 0, :])
                    nc.vector.tensor_mul(
                        out=tmp, in0=ov[0:HD, 1, :], in1=bc_sb[:, 1, :])
                    nc.scalar.dma_start(outT[HD:128, j, :], tmp)
                    nc.vector.tensor_mul(
                        out=outT[0:HD, j, :], in0=ov[0:HD, 0, :],
                        in1=bc_sb[:, 0, :])

                # first-half projection (chunks 0-3, ready after pair 3)
                # interleaved into pairs 4-7; partial sums + bias staged to
                # SBUF so the tail only needs the cc4-7 matmuls + one add
                for m in {4: [0], 5: [1], 6: [2], 7: [3]}.get(j, []):
                    pp = ov_ps.tile([128, 2, 512], F32, tag="ov")
                    for jn in range(2):
                        for cc in range(4):
                            nc.tensor.matmul(
                                pp[:, jn, :],
                                lhsT=outT[:, cc, m * 128:(m + 1) * 128],
                                rhs=sb_wht[:, cc, jn * 512:(jn + 1) * 512],
                                start=(cc == 0), stop=(cc == 3))
                    nc.vector.tensor_add(
                        out=p1sb[:, m, :, :], in0=pp, in1=sb_bias)

            # projection tail: remaining chunks 4-7 + staged first half.
            # One [128,2,512] tile per q-row-block (both jn halves), groups
            # alternating between the two PSUM pools so all four are open at
            # once -> chunk-4..6 matmuls run while the last pair normalizes.
            for m in range(QBLK // 128):
                pool = scps if m % 2 == 0 else ov_ps
                tag = "S" if m % 2 == 0 else "ov"
                PT = pool.tile([128, 2, 512], F32, tag=tag)
                for jn in range(D // 512):
                    for cc in range(4, 8):
                        nc.tensor.matmul(
                            PT[:, jn, :],
                            lhsT=outT[:, cc, m * 128:(m + 1) * 128],
                            rhs=sb_wht[:, cc, jn * 512:(jn + 1) * 512],
                            start=(cc == 4), stop=(cc == 7))
                Fo = final_pool.tile([128, 2, 512], F32, tag="F")
                nc.vector.tensor_add(out=Fo, in0=PT, in1=p1sb[:, m, :, :])
                eng = nc.sync if m % 2 == 0 else nc.scalar
                eng.dma_start(out.ap()[m * 128:(m + 1) * 128, :], Fo)

        if niter == 1:
            body()
        else:
            with tc.For_i(
                0, niter, 1,
                hint_engines=(
                    mybir.EngineType.PE,
                    mybir.EngineType.Activation,
                    mybir.EngineType.DVE,
                    mybir.EngineType.SP,
                    mybir.EngineType.Pool,
                ),
                staggered_reset=True,
            ) as _i:
                body(_i)

    nc.compile()
    return nc


def _host_shard(Q, K, V, W_H, b_H):
    """Build the 8 per-core input dicts (all host-side numpy)."""
    from ml_dtypes import bfloat16

    Qs = (np.asarray(Q, np.float32) * SCALE)
    K = np.asarray(K, np.float32)
    V = np.asarray(V, np.float32)
    W_H = np.asarray(W_H, np.float32)
    b_H = np.asarray(b_H, np.float32)

    # [hd, n] chunked: [128, 8, D]
    wht = np.ascontiguousarray(
        W_H.T.reshape(8, 128, D).transpose(1, 0, 2)).astype(bfloat16)
    bias = np.ascontiguousarray(np.broadcast_to(b_H, (128, D))).astype(np.float32)

    in_maps = []
    for c in range(NCORES):
        b, half = divmod(c, 2)
        qlo = half * QBLK
        # [q, j, par, d] -> [par, d, j, q] -> [128, 8, QBLK]
        qtc = np.ascontiguousarray(
            Qs[b, qlo:qlo + QBLK].reshape(QBLK, 8, 2, HD).transpose(2, 3, 1, 0)
        ).reshape(128, 8, QBLK).astype(bfloat16)
        ktc = np.ascontiguousarray(
            K[b].reshape(L, 8, 2, HD).transpose(2, 3, 1, 0)
        ).reshape(128, 8, L).astype(bfloat16)
        # V_aug [k, h, 65] -> [c, p, j, par, e] -> [p, j, par, c, e]
        va = np.concatenate(
            [V[b].reshape(L, H, HD), np.ones((L, H, 1), np.float32)], axis=2)
        vvc = np.ascontiguousarray(
            va.reshape(8, 128, 8, 2, HD + 1).transpose(1, 2, 3, 0, 4)
        ).astype(bfloat16)
        in_maps.append({"qt": qtc, "kt": ktc, "vv": vvc, "wht": wht,
                        "bias": bias})
    return in_maps


def _get_runner(niter=1):
    """Build (once) and cache a jitted 8-core runner for the kernel."""
    key = ("runner", niter)
    if key in _STATE:
        return _STATE[key]

    import jax
    from jax.sharding import Mesh, PartitionSpec, NamedSharding
    from jax.experimental.shard_map import shard_map
    from concourse import bass2jax, mybir

    nc = _build_nc(niter)
    bass2jax.install_neuronx_cc_hook()

    partition_name = (
        nc.partition_id_tensor.name if nc.partition_id_tensor else None)
    in_names, out_names, out_avals, zero_shapes = [], [], [], []
    for alloc in nc.m.functions[0].allocations:
        if not isinstance(alloc, mybir.MemoryLocationSet):
            continue
        name = alloc.memorylocations[0].name
        if alloc.kind == "ExternalInput":
            if name != partition_name:
                in_names.append(name)
        elif alloc.kind == "ExternalOutput":
            out_names.append(name)
            shape = tuple(alloc.tensor_shape)
            dtype = mybir.dt.np(alloc.dtype)
            out_avals.append(jax.core.ShapedArray(shape, dtype))
            zero_shapes.append((shape, dtype))
    n_params = len(in_names)
    n_outs = len(out_avals)
    all_names = list(in_names) + list(out_names)
    if partition_name is not None:
        all_names.append(partition_name)
    donate = tuple(range(n_params, n_params + n_outs))

    def _body(*args):
        operands = list(args)
        if partition_name is not None:
            operands.append(bass2jax.partition_id_tensor())
        outs = bass2jax._bass_exec_p.bind(
            *operands,
            out_avals=tuple(out_avals),
            in_names=tuple(all_names),
            out_names=tuple(out_names),
            lowering_input_output_aliases=(),
            sim_require_finite=True,
            sim_require_nnan=True,
            nc=nc,
        )
        return tuple(outs)

    devices = jax.devices()[:NCORES]
    mesh = Mesh(np.asarray(devices), ("core",))
    in_specs = (PartitionSpec("core"),) * (n_params + n_outs)
    out_specs = (PartitionSpec("core"),) * n_outs
    sharded = jax.jit(
        shard_map(_body, mesh=mesh, in_specs=in_specs, out_specs=out_specs,
                  check_rep=False),
        donate_argnums=donate,
        keep_unused=True,
    )
    sharding = NamedSharding(mesh, PartitionSpec("core"))

    def put_inputs(in_maps):
        return [
            jax.device_put(
                np.concatenate(
                    [np.asarray(in_maps[c][nm]) for c in range(NCORES)], axis=0),
                sharding)
            for nm in in_names
        ]

    def run(in_maps, device_inputs=None):
        if device_inputs is None:
            device_inputs = put_inputs(in_maps)
        zeros = [
            jax.device_put(np.zeros((NCORES * s[0], *s[1:]), d), sharding)
            for s, d in zero_shapes
        ]
        out_arrs = sharded(*device_inputs, *zeros)
        results = []
        for c in range(NCORES):
            results.append({
                name: np.asarray(out_arrs[i]).reshape(
                    NCORES, *out_avals[i].shape)[c]
                for i, name in enumerate(out_names)
            })
        return results

    runner = {"run": run, "put_inputs": put_inputs, "sharded": sharded,
              "in_names": in_names, "out_names": out_names,
              "zero_shapes": zero_shapes, "nc": nc}
    _STATE[key] = runner
    return runner


def kernel(Q=None, K=None, V=None, W_H=None, b_H=None, mask=None, **kw):
    in_maps = _host_shard(Q, K, V, W_H, b_H)
    runner = _get_runner(niter=1)
    results = runner["run"](in_maps)
    out = np.empty((B, L, D), np.float32)
    for c in range(NCORES):
        b, half = divmod(c, 2)
        out[b, half * QBLK:(half + 1) * QBLK, :] = results[c]["out"]
    return out
